# revision 37
# baseline (speedup 1.0000x reference)
"""Trainium2 Bass kernel for nn_Attention (S=2048, B=2, D=1024, H=16, C=64).

Tensor-parallel over heads across 8 NeuronCores (2 heads/core):
  - host passes x pre-transposed (xT [D, T], batch-major tokens t = b*S + s),
    per-core column slices of Wq/Wk/Wv, full Wout, and rope factor tables.
  - device: qT/kT = (W_local.T @ xT) with fused RMSNorm (partition-reduction
    via matmul-with-ones) and RoPE (elementwise, partner rows via 32-aligned
    partition slices); v via PE transpose of vT, augmented with a ones column
    so the softmax denominator falls out of the attn@v matmul.
  - scores computed transposed ([keys, queries]) so softmax needs no
    transposes; exp on ScalarE reading PSUM directly.
  - AllToAll re-shards from head-cols to token-rows; each core then computes
    its 512-token slice of the output projection against the full Wout.
Matmuls run as float32r (full-rate fp32, ~1e-4 rel err).
"""

import sys

if "/opt/trn_rl_repo" not in sys.path:
    sys.path.insert(0, "/opt/trn_rl_repo")

import numpy as np
import concourse.bass as bass
from concourse import bacc, tile, mybir
from concourse.bass_utils import run_bass_kernel_spmd
from concourse.masks import make_identity

S, B, D, H, C = 2048, 2, 1024, 16, 64
EPS = 1e-6
NCORES = 8
T = S * B                  # 4096 tokens, batch-major: t = b*S + s
LH = H // NCORES           # 2 local heads
LC = LH * C                # 128 local head columns
TCH = 512                  # phase-1 token chunk
NCHUNK = T // TCH          # 8
ICH = 1024                 # phase-2 query chunk (per batch)
TOK_OUT = T // NCORES      # 512 output tokens per core

F32 = mybir.dt.float32
F32R = mybir.dt.float32r
BF16 = mybir.dt.bfloat16
AF = mybir.ActivationFunctionType

_CACHE = {}
LAST_RESULTS = None


def r(ap):
    return ap


def _build():
    nc = bacc.Bacc("TRN2", target_bir_lowering=False, debug=False,
                   num_devices=NCORES)
    xT = nc.dram_tensor("xT", [D, T], F32, kind="ExternalInput")
    wq = nc.dram_tensor("wq", [D, LC], F32, kind="ExternalInput")
    wk = nc.dram_tensor("wk", [D, LC], F32, kind="ExternalInput")
    wv = nc.dram_tensor("wv", [D, LC], F32, kind="ExternalInput")
    wo = nc.dram_tensor("wo", [H * C, D], F32, kind="ExternalInput")
    ropeA = nc.dram_tensor("ropeA", [LC, T], F32, kind="ExternalInput")
    ropeB = nc.dram_tensor("ropeB", [LC, T], F32, kind="ExternalInput")
    qw = nc.dram_tensor("qw", [C], F32, kind="ExternalInput")
    kw = nc.dram_tensor("kw", [C], F32, kind="ExternalInput")
    out = nc.dram_tensor("out", [TOK_OUT, D], F32, kind="ExternalOutput")

    xT4 = xT.rearrange("(a p) t -> p a t", p=128)       # [128, 8, T]
    wq4 = wq.rearrange("(a p) c -> p a c", p=128)       # [128, 8, LC]
    wk4 = wk.rearrange("(a p) c -> p a c", p=128)
    wv4 = wv.rearrange("(a p) c -> p a c", p=128)
    wo4 = wo.rearrange("(a p) n -> p a n", p=128)       # [128, 8, D]

    with tile.TileContext(nc) as tc:
        with (
            tc.tile_pool(name="singles", bufs=1) as singles,
            tc.tile_pool(name="xtp", bufs=4) as xtp,
            tc.tile_pool(name="ropep", bufs=2) as ropep,
            tc.tile_pool(name="workp", bufs=3) as workp,
            tc.tile_pool(name="expp", bufs=2) as expp,
            tc.tile_pool(name="outp", bufs=2) as outp,
            tc.tile_pool(name="ps_big", bufs=2, space="PSUM") as ps_big,
            tc.tile_pool(name="ps_op", bufs=2, space="PSUM") as ps_op,
            tc.tile_pool(name="dram", bufs=1, space="DRAM") as dram,
        ):
            # ---- constants ----
            ident = singles.tile([128, 128], F32)
            make_identity(nc, ident)
            ones2f = singles.tile([128, 2], F32)
            nc.vector.memset(ones2f, 0.0)
            nc.vector.memset(ones2f[0:64, 0:1], 1.0)
            nc.vector.memset(ones2f[64:128, 1:2], 1.0)
            ones2 = singles.tile([128, 2], F32R)
            nc.vector.tensor_copy(ones2, ones2f)
            eps2 = singles.tile([2, 1], F32)
            nc.vector.memset(eps2, EPS)
            eps128 = singles.tile([128, 1], F32)
            nc.vector.memset(eps128, EPS)
            eps64 = singles.tile([128, 1], F32)
            nc.vector.memset(eps64, C * EPS)

            # ---- weights ----
            wq_sb = singles.tile([128, 8, LC], BF16)
            wk_sb = singles.tile([128, 8, LC], BF16)
            wv_sb = singles.tile([128, 8, LC], BF16)
            nc.gpsimd.dma_start(out=wq_sb, in_=wq4)
            nc.gpsimd.dma_start(out=wk_sb, in_=wk4)
            nc.gpsimd.dma_start(out=wv_sb, in_=wv4)

            # ---- persistent activations ----
            kRstd = singles.tile([128, T // 128, LH], F32)  # rstd_k/8 per token
            qw128 = singles.tile([128, 1], F32)
            nc.sync.dma_start(out=qw128[0:64, :], in_=qw[:, None])
            nc.sync.dma_start(out=qw128[64:128, :], in_=qw[:, None])
            kw128 = singles.tile([128, 1], F32)
            nc.sync.dma_start(out=kw128[0:64, :], in_=kw[:, None])
            nc.sync.dma_start(out=kw128[64:128, :], in_=kw[:, None])
            qw128 = singles.tile([128, 1], F32)
            nc.sync.dma_start(out=qw128[0:64, :], in_=qw[:, None])
            nc.sync.dma_start(out=qw128[64:128, :], in_=qw[:, None])
            qT_sb = singles.tile([128, T], BF16)          # [c_local, t]
            kTz0 = singles.tile([128, T], BF16)   # head0 rows 0:64, rest 0
            kTz1 = singles.tile([128, T], BF16)   # head1 rows 64:128, rest 0
            nc.vector.memset(kTz0[64:128, :], 0.0)
            nc.vector.memset(kTz1[0:64, :], 0.0)
            v_sb = singles.tile([128, T // 128, 2 * (C + 1)], BF16)  # [t%128, t//128, 130]
            onescol = singles.tile([128, T // 128, 1], F32)
            nc.vector.memset(onescol, 1.0)
            nc.vector.tensor_copy(v_sb[:, :, C:C + 1], onescol)
            nc.vector.tensor_copy(v_sb[:, :, 2 * C + 1:2 * C + 2], onescol)
            bounce_in_a = dram.tile([NCORES * 128, TOK_OUT // 2], BF16)
            bounce_in_b = dram.tile([NCORES * 128, TOK_OUT // 2], BF16)

            # ================= phase 1: projections + norm + rope =========
            for ch in range(NCHUNK):
                t0 = ch * TCH
                xt = xtp.tile([128, 8, TCH], BF16, tag="xt")
                nc.gpsimd.dma_start(out=xt, in_=xT4[:, :, t0:t0 + TCH])
                ra = ropep.tile([128, TCH], F32, tag="ra")
                rb = ropep.tile([128, TCH], F32, tag="rb")
                nc.sync.dma_start(out=ra, in_=ropeA[:, t0:t0 + TCH])
                nc.sync.dma_start(out=rb, in_=ropeB[:, t0:t0 + TCH])

                # ---- all three projections first (dense PE stream) ----
                psq = ps_big.tile([128, TCH], F32, tag="mm2")
                for a in range(8):
                    nc.tensor.matmul(psq, r(wq_sb[:, a, :]), r(xt[:, a, :]),
                                     start=(a == 0), stop=(a == 7))
                psk = ps_big.tile([128, TCH], F32, tag="mm2")
                for a in range(8):
                    nc.tensor.matmul(psk, r(wk_sb[:, a, :]), r(xt[:, a, :]),
                                     start=(a == 0), stop=(a == 7))
                psv = ps_op.tile([128, TCH], F32, tag="po")
                for a in range(8):
                    nc.tensor.matmul(psv, r(wv_sb[:, a, :]), r(xt[:, a, :]),
                                     start=(a == 0), stop=(a == 7))
                qraw = workp.tile([128, TCH], F32, tag="qraw")
                nc.scalar.copy(qraw, psq)
                kraw = workp.tile([128, TCH], F32, tag="kraw")
                nc.scalar.copy(kraw, psk)
                vt = workp.tile([128, TCH], F32, tag="vt")
                nc.scalar.copy(vt, psv)

                # ---- q: norm via sumsq matmul + DRAM-reshaped recip ----
                sq = workp.tile([128, TCH], F32R, tag="sq")
                nc.gpsimd.tensor_mul(sq, qraw, qraw)
                ms = ps_op.tile([2, TCH], F32, tag="po")
                nc.tensor.matmul(ms, r(ones2), r(sq), start=True, stop=True)
                rstd = workp.tile([2, TCH], F32, tag="rstd")
                nc.scalar.activation(rstd, ms, AF.Sqrt, bias=eps2,
                                     scale=1.0 / C)
                sq_dr = dram.tile([2, TCH], F32, tag="sq_dr", bufs=2)
                nc.sync.dma_start(out=sq_dr, in_=rstd)
                rsh = workp.tile([64, 2 * TCH // 64], F32, tag="rsh")
                nc.sync.dma_start(
                    out=rsh,
                    in_=bass.AP(tensor=sq_dr.tensor, offset=sq_dr.offset,
                                ap=[[2 * TCH // 64, 64], [1, 2 * TCH // 64]]))
                rshr = workp.tile([64, 2 * TCH // 64], F32, tag="rshr")
                nc.vector.reciprocal(rshr, rsh)
                rstd_dr = dram.tile([2, TCH], F32, tag="rstd_dr", bufs=2)
                nc.sync.dma_start(
                    out=bass.AP(tensor=rstd_dr.tensor, offset=rstd_dr.offset,
                                ap=[[2 * TCH // 64, 64], [1, 2 * TCH // 64]]),
                    in_=rshr)
                wbs = workp.tile([128, TCH], F32, tag="wbs")
                nc.sync.dma_start(
                    out=wbs,
                    in_=bass.AP(tensor=rstd_dr.tensor, offset=rstd_dr.offset,
                                ap=[[TCH, 2], [0, 64], [1, TCH]]))
                qw_s = workp.tile([128, TCH], F32, tag="qn0")
                nc.vector.tensor_scalar_mul(qw_s, qraw, qw128)
                qn = workp.tile([128, TCH], F32, tag="qn")
                nc.gpsimd.tensor_mul(qn, qw_s, wbs)
                t1 = workp.tile([128, TCH], F32, tag="t1")
                nc.vector.tensor_mul(t1, ra, qn)
                rot = workp.tile([128, TCH], F32, tag="rot")
                for g0 in (0, 64):
                    nc.sync.dma_start(out=rot[g0:g0 + 32, :],
                                      in_=qn[g0 + 32:g0 + 64, :])
                    nc.sync.dma_start(out=rot[g0 + 32:g0 + 64, :],
                                      in_=qn[g0:g0 + 32, :])
                t2 = workp.tile([128, TCH], F32, tag="t2")
                nc.gpsimd.tensor_mul(t2, rb, rot)
                qf = workp.tile([128, TCH], F32, tag="qf")
                nc.vector.tensor_add(qf, t1, t2)
                nc.scalar.copy(qT_sb[:, t0:t0 + TCH], qf)

                # ---- k: rstd_k deferred to exp scale (commutes w/ rope) ----
                sqk = workp.tile([128, TCH], F32R, tag="sqk")
                nc.gpsimd.tensor_mul(sqk, kraw, kraw)
                mst = ps_op.tile([128, 2 * (TCH // 128)], F32, tag="po")
                for s5 in range(TCH // 128):
                    nc.tensor.matmul(mst[:, s5 * 2:(s5 + 1) * 2],
                                     r(sqk[:, s5 * 128:(s5 + 1) * 128]),
                                     r(ones2), start=True, stop=True)
                srt = workp.tile([128, 2 * (TCH // 128)], F32, tag="srt")
                nc.scalar.activation(srt, mst, AF.Sqrt, bias=eps64, scale=1.0)
                nc.vector.reciprocal(
                    kRstd[:, t0 // 128:(t0 + TCH) // 128, :]
                    .rearrange("p a b -> p (a b)"), srt)
                kw_s = workp.tile([128, TCH], F32, tag="kw_s")
                nc.vector.tensor_scalar_mul(kw_s, kraw, kw128)
                t1k = workp.tile([128, TCH], F32, tag="t1")
                nc.vector.tensor_mul(t1k, ra, kw_s)
                rotk = workp.tile([128, TCH], F32, tag="rot")
                for g0 in (0, 64):
                    nc.sync.dma_start(out=rotk[g0:g0 + 32, :],
                                      in_=kw_s[g0 + 32:g0 + 64, :])
                    nc.sync.dma_start(out=rotk[g0 + 32:g0 + 64, :],
                                      in_=kw_s[g0:g0 + 32, :])
                t2k = workp.tile([128, TCH], F32, tag="t2")
                nc.gpsimd.tensor_mul(t2k, rb, rotk)
                kf = workp.tile([128, TCH], F32, tag="qf")
                nc.vector.tensor_add(kf, t1k, t2k)
                nc.scalar.copy(kTz0[0:64, t0:t0 + TCH], kf[0:64, :])
                nc.scalar.copy(kTz1[64:128, t0:t0 + TCH], kf[64:128, :])

                # ---- v: transpose to [t, c] with aug ones columns ----
                for s5 in range(TCH // 128):
                    pt = ps_op.tile([128, 128], F32, tag="po")
                    nc.tensor.transpose(pt, vt[:, s5 * 128:(s5 + 1) * 128],
                                        ident)
                    tt = (t0 + s5 * 128) // 128
                    nc.vector.tensor_copy(v_sb[:, tt, 0:C], pt[:, 0:C])
                    nc.vector.tensor_copy(v_sb[:, tt, C + 1:2 * C + 1],
                                          pt[:, C:2 * C])

            wo_a = xtp.tile([128, 8, 512], BF16, tag="xt")
            wo_b = xtp.tile([128, 8, 512], BF16, tag="xt")
            nc.gpsimd.dma_start(out=wo_a, in_=wo4[:, :, 0:512])
            nc.gpsimd.dma_start(out=wo_b, in_=wo4[:, :, 512:1024])

            # ================= phase 2: attention ========================
            for b in range(B):
                for ic in range(S // ICH):
                    q0 = b * S + ic * ICH
                    pos = [ps_op.tile([C + 1, ICH], F32, tag="po",
                                      name=f"po_{b}_{ic}_{lh}")
                           for lh in range(LH)]
                    for jt in range(S // 128):
                        j0 = b * S + jt * 128
                        pss = [ps_big.tile([128, ICH], F32, tag="mm2",
                                           name=f"pss_{b}_{ic}_{jt}_{lh}")
                               for lh in range(LH)]
                        for hf in range(2):
                            for lh in range(LH):
                                ktz = kTz0 if lh == 0 else kTz1
                                nc.tensor.matmul(
                                    pss[lh][:, hf * 512:(hf + 1) * 512],
                                    r(ktz[:, j0:j0 + 128]),
                                    r(qT_sb[:,
                                            q0 + hf * 512:q0 + (hf + 1) * 512]),
                                    start=True, stop=True)
                        exs = []
                        for lh in range(LH):
                            ex = expp.tile([128, ICH], BF16, tag="ex",
                                           name=f"ex_{b}_{ic}_{jt}_{lh}")
                            nc.scalar.activation(
                                ex, pss[lh], AF.Exp, bias=0.0,
                                scale=kRstd[:, j0 // 128, lh:lh + 1])
                            exs.append(ex)
                        for hf in range(2):
                            for lh in range(LH):
                                nc.tensor.matmul(
                                    pos[lh][:, hf * 512:(hf + 1) * 512],
                                    r(v_sb[:, j0 // 128,
                                           lh * (C + 1):(lh + 1) * (C + 1)]),
                                    r(exs[lh][:, hf * 512:(hf + 1) * 512]),
                                    start=(jt == 0), stop=(jt == S // 128 - 1))
                    for lh in range(LH):
                        hr = C * lh
                        po = pos[lh]
                        po_sb = workp.tile([C + 1, ICH], F32, tag="po_sb")
                        nc.vector.tensor_copy(po_sb, po)
                        dn_dr = dram.tile([1, ICH], F32, tag="dn_dr",
                                          bufs=2)
                        nc.sync.dma_start(out=dn_dr, in_=po_sb[C:C + 1, :])
                        dnsh = workp.tile([64, ICH // 64], F32, tag="rsh")
                        nc.sync.dma_start(
                            out=dnsh,
                            in_=bass.AP(tensor=dn_dr.tensor, offset=dn_dr.offset,
                                        ap=[[ICH // 64, 64], [1, ICH // 64]]))
                        dnshr = workp.tile([64, ICH // 64], F32, tag="rshr")
                        nc.vector.reciprocal(dnshr, dnsh)
                        den_dr = dram.tile([1, ICH], F32, tag="den_dr",
                                           bufs=2)
                        nc.sync.dma_start(
                            out=bass.AP(tensor=den_dr.tensor,
                                        offset=den_dr.offset,
                                        ap=[[ICH // 64, 64], [1, ICH // 64]]),
                            in_=dnshr)
                        nrm = workp.tile([C, ICH], F32, tag="wbs")
                        nc.sync.dma_start(
                            out=nrm,
                            in_=bass.AP(tensor=den_dr.tensor,
                                        offset=den_dr.offset,
                                        ap=[[0, C], [1, ICH]]))
                        att_tmp = workp.tile([C, ICH], BF16, tag="att_tmp")
                        nc.gpsimd.tensor_mul(att_tmp, po_sb[0:C, :], nrm)
                        for hf in range(2):
                            g2 = (q0 + hf * 512) // TOK_OUT
                            for bb, bnc in ((0, bounce_in_a), (1, bounce_in_b)):
                                nc.sync.dma_start(
                                    out=bnc[g2 * 128 + hr:g2 * 128 + hr + C,
                                            :],
                                    in_=att_tmp[:, hf * 512 + bb * 256:
                                                hf * 512 + (bb + 1) * 256])

            # ================= phase 3: all-to-all + out projection =======
            bounce_out_a = dram.tile([NCORES * 128, TOK_OUT // 2], BF16)
            bounce_out_b = dram.tile([NCORES * 128, TOK_OUT // 2], BF16)
            HT = TOK_OUT // 2
            for half, (bi, bo) in enumerate(((bounce_in_a, bounce_out_a),
                                             (bounce_in_b, bounce_out_b))):
                nc.gpsimd.collective_compute(
                    "AllToAll", mybir.AluOpType.bypass,
                    replica_groups=[list(range(NCORES))],
                    ins=[bi[:, :].opt()],
                    outs=[bo[:, :].opt()])
                att_h = xtp.tile([128, 8, HT], BF16, tag="xt",
                                 name=f"att_h{half}")
                nc.sync.dma_start(
                    out=att_h,
                    in_=bo.rearrange("(g p) t -> p g t", p=128))
                for ts2 in range(HT // 128):
                    ts = half * (HT // 128) + ts2
                    out_sb = outp.tile([128, D], F32, tag="osb")
                    for nh in range(2):
                        pp = ps_big.tile([128, 512], F32, tag="mm2")
                        for a in range(8):
                            nc.tensor.matmul(
                                pp,
                                r(att_h[:, a, ts2 * 128:(ts2 + 1) * 128]),
                                r((wo_a if nh == 0 else wo_b)[:, a, :]),
                                start=(a == 0), stop=(a == 7))
                        nc.vector.tensor_copy(
                            out_sb[:, nh * 512:(nh + 1) * 512], pp)
                    nc.sync.dma_start(out=out[ts * 128:(ts + 1) * 128, :],
                                      in_=out_sb)

    nc.compile()
    return nc


def kernel(x, rope_emb, Wq, Wk, Wv, q_norm_w, k_norm_w, Wout):
    global LAST_RESULTS
    if "nc" not in _CACHE:
        _CACHE["nc"] = _build()
    nc = _CACHE["nc"]

    # batch-major tokens: t = b*S + s
    x2 = np.ascontiguousarray(
        np.transpose(np.asarray(x, np.float32), (1, 0, 2)).reshape(T, D))
    xT_np = np.ascontiguousarray(x2.T)

    re = np.asarray(rope_emb, np.float32)
    cosT = np.ascontiguousarray(re[:, :, 0, 0].T)    # [32, S]
    r01T = np.ascontiguousarray(re[:, :, 0, 1].T)
    r10T = np.ascontiguousarray(re[:, :, 1, 0].T)
    cos2 = np.concatenate([cosT, cosT], axis=1)      # [32, T] batch-major
    r01_2 = np.concatenate([r01T, r01T], axis=1)
    r10_2 = np.concatenate([r10T, r10T], axis=1)
    ropeA_np = np.ascontiguousarray(
        np.concatenate([cos2, cos2, cos2, cos2], axis=0))
    ropeB_np = np.ascontiguousarray(
        np.concatenate([r01_2, r10_2, r01_2, r10_2], axis=0))

    Wq = np.asarray(Wq, np.float32)
    Wk = np.asarray(Wk, np.float32)
    Wv = np.asarray(Wv, np.float32)
    Wout = np.ascontiguousarray(np.asarray(Wout, np.float32))
    qw_np = np.ascontiguousarray(np.asarray(q_norm_w, np.float32))
    kw_np = np.ascontiguousarray(np.asarray(k_norm_w, np.float32))

    in_maps = []
    for g in range(NCORES):
        sl = slice(g * LC, (g + 1) * LC)
        in_maps.append({
            "xT": xT_np,
            "wq": np.ascontiguousarray(Wq[:, sl]),
            "wk": np.ascontiguousarray(Wk[:, sl]),
            "wv": np.ascontiguousarray(Wv[:, sl]),
            "wo": Wout,
            "ropeA": ropeA_np,
            "ropeB": ropeB_np,
            "qw": qw_np,
            "kw": kw_np,
        })

    res = run_bass_kernel_spmd(nc, in_maps, core_ids=list(range(NCORES)))
    LAST_RESULTS = res
    out_full = np.concatenate([res.results[g]["out"] for g in range(NCORES)],
                              axis=0)                 # [T, D] batch-major
    return np.ascontiguousarray(
        out_full.reshape(B, S, D).transpose(1, 0, 2))


# revision 38
# speedup vs baseline: 1.0147x; 1.0147x over previous
"""Trainium2 Bass kernel for nn_Attention (S=2048, B=2, D=1024, H=16, C=64).

Tensor-parallel over heads across 8 NeuronCores (2 heads/core):
  - host passes x pre-transposed (xT [D, T], batch-major tokens t = b*S + s),
    per-core column slices of Wq/Wk/Wv, full Wout, and rope factor tables.
  - device: qT/kT = (W_local.T @ xT) with fused RMSNorm (partition-reduction
    via matmul-with-ones) and RoPE (elementwise, partner rows via 32-aligned
    partition slices); v via PE transpose of vT, augmented with a ones column
    so the softmax denominator falls out of the attn@v matmul.
  - scores computed transposed ([keys, queries]) so softmax needs no
    transposes; exp on ScalarE reading PSUM directly.
  - AllToAll re-shards from head-cols to token-rows; each core then computes
    its 512-token slice of the output projection against the full Wout.
Matmuls run as float32r (full-rate fp32, ~1e-4 rel err).
"""

import sys

if "/opt/trn_rl_repo" not in sys.path:
    sys.path.insert(0, "/opt/trn_rl_repo")

import numpy as np
import concourse.bass as bass
from concourse import bacc, tile, mybir
from concourse.bass_utils import run_bass_kernel_spmd
from concourse.masks import make_identity

S, B, D, H, C = 2048, 2, 1024, 16, 64
EPS = 1e-6
NCORES = 8
T = S * B                  # 4096 tokens, batch-major: t = b*S + s
LH = H // NCORES           # 2 local heads
LC = LH * C                # 128 local head columns
TCH = 512                  # phase-1 token chunk
NCHUNK = T // TCH          # 8
ICH = 1024                 # phase-2 query chunk (per batch)
TOK_OUT = T // NCORES      # 512 output tokens per core

F32 = mybir.dt.float32
F32R = mybir.dt.float32r
BF16 = mybir.dt.bfloat16
AF = mybir.ActivationFunctionType

_CACHE = {}
LAST_RESULTS = None


def r(ap):
    return ap


def _build():
    nc = bacc.Bacc("TRN2", target_bir_lowering=False, debug=False,
                   num_devices=NCORES)
    xT = nc.dram_tensor("xT", [D, T], F32, kind="ExternalInput")
    wq = nc.dram_tensor("wq", [D, LC], F32, kind="ExternalInput")
    wk = nc.dram_tensor("wk", [D, LC], F32, kind="ExternalInput")
    wv = nc.dram_tensor("wv", [D, LC], F32, kind="ExternalInput")
    wo = nc.dram_tensor("wo", [H * C, D], F32, kind="ExternalInput")
    ropeA = nc.dram_tensor("ropeA", [LC, T], F32, kind="ExternalInput")
    ropeB = nc.dram_tensor("ropeB", [LC, T], F32, kind="ExternalInput")
    qw = nc.dram_tensor("qw", [C], F32, kind="ExternalInput")
    kw = nc.dram_tensor("kw", [C], F32, kind="ExternalInput")
    out = nc.dram_tensor("out", [TOK_OUT, D], F32, kind="ExternalOutput")

    xT4 = xT.rearrange("(a p) t -> p a t", p=128)       # [128, 8, T]
    wq4 = wq.rearrange("(a p) c -> p a c", p=128)       # [128, 8, LC]
    wk4 = wk.rearrange("(a p) c -> p a c", p=128)
    wv4 = wv.rearrange("(a p) c -> p a c", p=128)
    wo4 = wo.rearrange("(a p) n -> p a n", p=128)       # [128, 8, D]

    with tile.TileContext(nc) as tc:
        with (
            tc.tile_pool(name="singles", bufs=1) as singles,
            tc.tile_pool(name="xtp", bufs=4) as xtp,
            tc.tile_pool(name="ropep", bufs=2) as ropep,
            tc.tile_pool(name="workp", bufs=3) as workp,
            tc.tile_pool(name="expp", bufs=2) as expp,
            tc.tile_pool(name="outp", bufs=2) as outp,
            tc.tile_pool(name="ps_big", bufs=2, space="PSUM") as ps_big,
            tc.tile_pool(name="ps_op", bufs=2, space="PSUM") as ps_op,
            tc.tile_pool(name="dram", bufs=1, space="DRAM") as dram,
        ):
            # ---- constants ----
            ident = singles.tile([128, 128], F32)
            make_identity(nc, ident)
            ones2f = singles.tile([128, 2], F32)
            nc.vector.memset(ones2f, 0.0)
            nc.vector.memset(ones2f[0:64, 0:1], 1.0)
            nc.vector.memset(ones2f[64:128, 1:2], 1.0)
            ones2 = singles.tile([128, 2], F32R)
            nc.vector.tensor_copy(ones2, ones2f)
            eps2 = singles.tile([2, 1], F32)
            nc.vector.memset(eps2, EPS)
            eps128 = singles.tile([128, 1], F32)
            nc.vector.memset(eps128, EPS)
            eps64 = singles.tile([128, 1], F32)
            nc.vector.memset(eps64, C * EPS)

            # ---- weights ----
            wq_sb = singles.tile([128, 8, LC], BF16)
            wk_sb = singles.tile([128, 8, LC], BF16)
            wv_sb = singles.tile([128, 8, LC], BF16)
            nc.gpsimd.dma_start(out=wq_sb, in_=wq4)
            nc.gpsimd.dma_start(out=wk_sb, in_=wk4)
            nc.gpsimd.dma_start(out=wv_sb, in_=wv4)

            # ---- persistent activations ----
            kRstd = singles.tile([128, T // 128, LH], F32)  # rstd_k/8 per token
            qw128 = singles.tile([128, 1], F32)
            nc.sync.dma_start(out=qw128[0:64, :], in_=qw[:, None])
            nc.sync.dma_start(out=qw128[64:128, :], in_=qw[:, None])
            kw128 = singles.tile([128, 1], F32)
            nc.sync.dma_start(out=kw128[0:64, :], in_=kw[:, None])
            nc.sync.dma_start(out=kw128[64:128, :], in_=kw[:, None])
            qw128 = singles.tile([128, 1], F32)
            nc.sync.dma_start(out=qw128[0:64, :], in_=qw[:, None])
            nc.sync.dma_start(out=qw128[64:128, :], in_=qw[:, None])
            qT_sb = singles.tile([128, T], BF16)          # [c_local, t]
            kTz0 = singles.tile([128, T], BF16)   # head0 rows 0:64, rest 0
            kTz1 = singles.tile([128, T], BF16)   # head1 rows 64:128, rest 0
            nc.vector.memset(kTz0[64:128, :], 0.0)
            nc.vector.memset(kTz1[0:64, :], 0.0)
            v_sb = singles.tile([128, T // 128, 2 * (C + 1)], BF16)  # [t%128, t//128, 130]
            onescol = singles.tile([128, T // 128, 1], F32)
            nc.vector.memset(onescol, 1.0)
            nc.vector.tensor_copy(v_sb[:, :, C:C + 1], onescol)
            nc.vector.tensor_copy(v_sb[:, :, 2 * C + 1:2 * C + 2], onescol)
            bounce_in = dram.tile([NCORES * 128, TOK_OUT], BF16)

            # ================= phase 1: projections + norm + rope =========
            for ch in range(NCHUNK):
                t0 = ch * TCH
                xt = xtp.tile([128, 8, TCH], BF16, tag="xt")
                nc.gpsimd.dma_start(out=xt, in_=xT4[:, :, t0:t0 + TCH])
                ra = ropep.tile([128, TCH], F32, tag="ra")
                rb = ropep.tile([128, TCH], F32, tag="rb")
                nc.sync.dma_start(out=ra, in_=ropeA[:, t0:t0 + TCH])
                nc.sync.dma_start(out=rb, in_=ropeB[:, t0:t0 + TCH])

                # ---- all three projections first (dense PE stream) ----
                psq = ps_big.tile([128, TCH], F32, tag="mm2")
                for a in range(8):
                    nc.tensor.matmul(psq, r(wq_sb[:, a, :]), r(xt[:, a, :]),
                                     start=(a == 0), stop=(a == 7))
                psk = ps_big.tile([128, TCH], F32, tag="mm2")
                for a in range(8):
                    nc.tensor.matmul(psk, r(wk_sb[:, a, :]), r(xt[:, a, :]),
                                     start=(a == 0), stop=(a == 7))
                psv = ps_op.tile([128, TCH], F32, tag="po")
                for a in range(8):
                    nc.tensor.matmul(psv, r(wv_sb[:, a, :]), r(xt[:, a, :]),
                                     start=(a == 0), stop=(a == 7))
                qraw = workp.tile([128, TCH], F32, tag="qraw")
                nc.scalar.copy(qraw, psq)
                kraw = workp.tile([128, TCH], F32, tag="kraw")
                nc.scalar.copy(kraw, psk)
                vt = workp.tile([128, TCH], F32, tag="vt")
                nc.scalar.copy(vt, psv)

                # ---- q: norm via sumsq matmul + DRAM-reshaped recip ----
                sq = workp.tile([128, TCH], F32R, tag="sq")
                nc.gpsimd.tensor_mul(sq, qraw, qraw)
                ms = ps_op.tile([2, TCH], F32, tag="po")
                nc.tensor.matmul(ms, r(ones2), r(sq), start=True, stop=True)
                rstd = workp.tile([2, TCH], F32, tag="rstd")
                nc.scalar.activation(rstd, ms, AF.Sqrt, bias=eps2,
                                     scale=1.0 / C)
                sq_dr = dram.tile([2, TCH], F32, tag="sq_dr", bufs=2)
                nc.sync.dma_start(out=sq_dr, in_=rstd)
                rsh = workp.tile([64, 2 * TCH // 64], F32, tag="rsh")
                nc.sync.dma_start(
                    out=rsh,
                    in_=bass.AP(tensor=sq_dr.tensor, offset=sq_dr.offset,
                                ap=[[2 * TCH // 64, 64], [1, 2 * TCH // 64]]))
                rshr = workp.tile([64, 2 * TCH // 64], F32, tag="rshr")
                nc.vector.reciprocal(rshr, rsh)
                rstd_dr = dram.tile([2, TCH], F32, tag="rstd_dr", bufs=2)
                nc.sync.dma_start(
                    out=bass.AP(tensor=rstd_dr.tensor, offset=rstd_dr.offset,
                                ap=[[2 * TCH // 64, 64], [1, 2 * TCH // 64]]),
                    in_=rshr)
                wbs = workp.tile([128, TCH], F32, tag="wbs")
                nc.sync.dma_start(
                    out=wbs,
                    in_=bass.AP(tensor=rstd_dr.tensor, offset=rstd_dr.offset,
                                ap=[[TCH, 2], [0, 64], [1, TCH]]))
                qw_s = workp.tile([128, TCH], F32, tag="qn0")
                nc.vector.tensor_scalar_mul(qw_s, qraw, qw128)
                qn = workp.tile([128, TCH], F32, tag="qn")
                nc.gpsimd.tensor_mul(qn, qw_s, wbs)
                t1 = workp.tile([128, TCH], F32, tag="t1")
                nc.vector.tensor_mul(t1, ra, qn)
                rot = workp.tile([128, TCH], F32, tag="rot")
                for g0 in (0, 64):
                    nc.sync.dma_start(out=rot[g0:g0 + 32, :],
                                      in_=qn[g0 + 32:g0 + 64, :])
                    nc.sync.dma_start(out=rot[g0 + 32:g0 + 64, :],
                                      in_=qn[g0:g0 + 32, :])
                t2 = workp.tile([128, TCH], F32, tag="t2")
                nc.gpsimd.tensor_mul(t2, rb, rot)
                qf = workp.tile([128, TCH], F32, tag="qf")
                nc.vector.tensor_add(qf, t1, t2)
                nc.scalar.copy(qT_sb[:, t0:t0 + TCH], qf)

                # ---- k: rstd_k deferred to exp scale (commutes w/ rope) ----
                sqk = workp.tile([128, TCH], F32R, tag="sqk")
                nc.gpsimd.tensor_mul(sqk, kraw, kraw)
                mst = ps_op.tile([128, 2 * (TCH // 128)], F32, tag="po")
                for s5 in range(TCH // 128):
                    nc.tensor.matmul(mst[:, s5 * 2:(s5 + 1) * 2],
                                     r(sqk[:, s5 * 128:(s5 + 1) * 128]),
                                     r(ones2), start=True, stop=True)
                srt = workp.tile([128, 2 * (TCH // 128)], F32, tag="srt")
                nc.scalar.activation(srt, mst, AF.Sqrt, bias=eps64, scale=1.0)
                nc.vector.reciprocal(
                    kRstd[:, t0 // 128:(t0 + TCH) // 128, :]
                    .rearrange("p a b -> p (a b)"), srt)
                kw_s = workp.tile([128, TCH], F32, tag="kw_s")
                nc.vector.tensor_scalar_mul(kw_s, kraw, kw128)
                t1k = workp.tile([128, TCH], F32, tag="t1")
                nc.vector.tensor_mul(t1k, ra, kw_s)
                rotk = workp.tile([128, TCH], F32, tag="rot")
                for g0 in (0, 64):
                    nc.sync.dma_start(out=rotk[g0:g0 + 32, :],
                                      in_=kw_s[g0 + 32:g0 + 64, :])
                    nc.sync.dma_start(out=rotk[g0 + 32:g0 + 64, :],
                                      in_=kw_s[g0:g0 + 32, :])
                t2k = workp.tile([128, TCH], F32, tag="t2")
                nc.gpsimd.tensor_mul(t2k, rb, rotk)
                kf = workp.tile([128, TCH], F32, tag="qf")
                nc.vector.tensor_add(kf, t1k, t2k)
                nc.scalar.copy(kTz0[0:64, t0:t0 + TCH], kf[0:64, :])
                nc.scalar.copy(kTz1[64:128, t0:t0 + TCH], kf[64:128, :])

                # ---- v: transpose to [t, c] with aug ones columns ----
                for s5 in range(TCH // 128):
                    pt = ps_op.tile([128, 128], F32, tag="po")
                    nc.tensor.transpose(pt, vt[:, s5 * 128:(s5 + 1) * 128],
                                        ident)
                    tt = (t0 + s5 * 128) // 128
                    nc.vector.tensor_copy(v_sb[:, tt, 0:C], pt[:, 0:C])
                    nc.vector.tensor_copy(v_sb[:, tt, C + 1:2 * C + 1],
                                          pt[:, C:2 * C])

            wo_a = xtp.tile([128, 8, 512], BF16, tag="xt")
            wo_b = xtp.tile([128, 8, 512], BF16, tag="xt")
            nc.gpsimd.dma_start(out=wo_a, in_=wo4[:, :, 0:512])
            nc.gpsimd.dma_start(out=wo_b, in_=wo4[:, :, 512:1024])

            # ================= phase 2: attention ========================
            for b in range(B):
                for ic in range(S // ICH):
                    q0 = b * S + ic * ICH
                    pos = [ps_op.tile([C + 1, ICH], F32, tag="po",
                                      name=f"po_{b}_{ic}_{lh}")
                           for lh in range(LH)]
                    for jt in range(S // 128):
                        j0 = b * S + jt * 128
                        pss = [ps_big.tile([128, ICH], F32, tag="mm2",
                                           name=f"pss_{b}_{ic}_{jt}_{lh}")
                               for lh in range(LH)]
                        for hf in range(2):
                            for lh in range(LH):
                                ktz = kTz0 if lh == 0 else kTz1
                                nc.tensor.matmul(
                                    pss[lh][:, hf * 512:(hf + 1) * 512],
                                    r(ktz[:, j0:j0 + 128]),
                                    r(qT_sb[:,
                                            q0 + hf * 512:q0 + (hf + 1) * 512]),
                                    start=True, stop=True)
                        exs = []
                        for lh in range(LH):
                            ex = expp.tile([128, ICH], BF16, tag="ex",
                                           name=f"ex_{b}_{ic}_{jt}_{lh}")
                            nc.scalar.activation(
                                ex, pss[lh], AF.Exp, bias=0.0,
                                scale=kRstd[:, j0 // 128, lh:lh + 1])
                            exs.append(ex)
                        for hf in range(2):
                            for lh in range(LH):
                                nc.tensor.matmul(
                                    pos[lh][:, hf * 512:(hf + 1) * 512],
                                    r(v_sb[:, j0 // 128,
                                           lh * (C + 1):(lh + 1) * (C + 1)]),
                                    r(exs[lh][:, hf * 512:(hf + 1) * 512]),
                                    start=(jt == 0), stop=(jt == S // 128 - 1))
                    for lh in range(LH):
                        hr = C * lh
                        po = pos[lh]
                        po_sb = workp.tile([C + 1, ICH], F32, tag="po_sb")
                        nc.vector.tensor_copy(po_sb, po)
                        dn_dr = dram.tile([1, ICH], F32, tag="dn_dr",
                                          bufs=2)
                        nc.sync.dma_start(out=dn_dr, in_=po_sb[C:C + 1, :])
                        dnsh = workp.tile([64, ICH // 64], F32, tag="rsh")
                        nc.sync.dma_start(
                            out=dnsh,
                            in_=bass.AP(tensor=dn_dr.tensor, offset=dn_dr.offset,
                                        ap=[[ICH // 64, 64], [1, ICH // 64]]))
                        dnshr = workp.tile([64, ICH // 64], F32, tag="rshr")
                        nc.vector.reciprocal(dnshr, dnsh)
                        den_dr = dram.tile([1, ICH], F32, tag="den_dr",
                                           bufs=2)
                        nc.sync.dma_start(
                            out=bass.AP(tensor=den_dr.tensor,
                                        offset=den_dr.offset,
                                        ap=[[ICH // 64, 64], [1, ICH // 64]]),
                            in_=dnshr)
                        nrm = workp.tile([C, ICH], F32, tag="wbs")
                        nc.sync.dma_start(
                            out=nrm,
                            in_=bass.AP(tensor=den_dr.tensor,
                                        offset=den_dr.offset,
                                        ap=[[0, C], [1, ICH]]))
                        att_tmp = workp.tile([C, ICH], BF16, tag="att_tmp")
                        nc.gpsimd.tensor_mul(att_tmp, po_sb[0:C, :], nrm)
                        for hf in range(2):
                            g2 = (q0 + hf * 512) // TOK_OUT
                            nc.sync.dma_start(
                                out=bounce_in[g2 * 128 + hr:g2 * 128 + hr + C,
                                              :],
                                in_=att_tmp[:, hf * 512:(hf + 1) * 512])

            # ================= phase 3: all-to-all + out projection =======
            bounce_out = dram.tile([NCORES * 128, TOK_OUT], BF16)
            nc.gpsimd.collective_compute(
                "AllToAll", mybir.AluOpType.bypass,
                replica_groups=[list(range(NCORES))],
                ins=[bounce_in[:, :].opt()],
                outs=[bounce_out[:, :].opt()])

            att_all = xtp.tile([128, 8, TOK_OUT], BF16, tag="xt")
            nc.sync.dma_start(
                out=att_all,
                in_=bounce_out.rearrange("(g p) t -> p g t", p=128))
            for ts in range(TOK_OUT // 128):
                out_sb = outp.tile([128, D], F32, tag="osb")
                for nh in range(2):
                    pp = ps_big.tile([128, 512], F32, tag="mm2")
                    for a in range(8):
                        nc.tensor.matmul(
                            pp, r(att_all[:, a, ts * 128:(ts + 1) * 128]),
                            r((wo_a if nh == 0 else wo_b)[:, a, :]),
                            start=(a == 0), stop=(a == 7))
                    nc.vector.tensor_copy(out_sb[:, nh * 512:(nh + 1) * 512], pp)
                nc.sync.dma_start(out=out[ts * 128:(ts + 1) * 128, :],
                                  in_=out_sb)

    nc.compile()
    return nc


def kernel(x, rope_emb, Wq, Wk, Wv, q_norm_w, k_norm_w, Wout):
    global LAST_RESULTS
    if "nc" not in _CACHE:
        _CACHE["nc"] = _build()
    nc = _CACHE["nc"]

    # batch-major tokens: t = b*S + s
    x2 = np.ascontiguousarray(
        np.transpose(np.asarray(x, np.float32), (1, 0, 2)).reshape(T, D))
    xT_np = np.ascontiguousarray(x2.T)

    re = np.asarray(rope_emb, np.float32)
    cosT = np.ascontiguousarray(re[:, :, 0, 0].T)    # [32, S]
    r01T = np.ascontiguousarray(re[:, :, 0, 1].T)
    r10T = np.ascontiguousarray(re[:, :, 1, 0].T)
    cos2 = np.concatenate([cosT, cosT], axis=1)      # [32, T] batch-major
    r01_2 = np.concatenate([r01T, r01T], axis=1)
    r10_2 = np.concatenate([r10T, r10T], axis=1)
    ropeA_np = np.ascontiguousarray(
        np.concatenate([cos2, cos2, cos2, cos2], axis=0))
    ropeB_np = np.ascontiguousarray(
        np.concatenate([r01_2, r10_2, r01_2, r10_2], axis=0))

    Wq = np.asarray(Wq, np.float32)
    Wk = np.asarray(Wk, np.float32)
    Wv = np.asarray(Wv, np.float32)
    Wout = np.ascontiguousarray(np.asarray(Wout, np.float32))
    qw_np = np.ascontiguousarray(np.asarray(q_norm_w, np.float32))
    kw_np = np.ascontiguousarray(np.asarray(k_norm_w, np.float32))

    in_maps = []
    for g in range(NCORES):
        sl = slice(g * LC, (g + 1) * LC)
        in_maps.append({
            "xT": xT_np,
            "wq": np.ascontiguousarray(Wq[:, sl]),
            "wk": np.ascontiguousarray(Wk[:, sl]),
            "wv": np.ascontiguousarray(Wv[:, sl]),
            "wo": Wout,
            "ropeA": ropeA_np,
            "ropeB": ropeB_np,
            "qw": qw_np,
            "kw": kw_np,
        })

    res = run_bass_kernel_spmd(nc, in_maps, core_ids=list(range(NCORES)))
    LAST_RESULTS = res
    out_full = np.concatenate([res.results[g]["out"] for g in range(NCORES)],
                              axis=0)                 # [T, D] batch-major
    return np.ascontiguousarray(
        out_full.reshape(B, S, D).transpose(1, 0, 2))


# revision 39
# speedup vs baseline: 1.0836x; 1.0679x over previous
"""Trainium2 Bass kernel for nn_Attention (S=2048, B=2, D=1024, H=16, C=64).

Tensor-parallel over heads across 8 NeuronCores (2 heads/core):
  - host passes x pre-transposed (xT [D, T], batch-major tokens t = b*S + s),
    per-core column slices of Wq/Wk/Wv, full Wout, and rope factor tables.
  - device: qT/kT = (W_local.T @ xT) with fused RMSNorm (partition-reduction
    via matmul-with-ones) and RoPE (elementwise, partner rows via 32-aligned
    partition slices); v via PE transpose of vT, augmented with a ones column
    so the softmax denominator falls out of the attn@v matmul.
  - scores computed transposed ([keys, queries]) so softmax needs no
    transposes; exp on ScalarE reading PSUM directly.
  - AllToAll re-shards from head-cols to token-rows; each core then computes
    its 512-token slice of the output projection against the full Wout.
Matmuls run as float32r (full-rate fp32, ~1e-4 rel err).
"""

import sys

if "/opt/trn_rl_repo" not in sys.path:
    sys.path.insert(0, "/opt/trn_rl_repo")

import numpy as np
import concourse.bass as bass
from concourse import bacc, tile, mybir
from concourse.bass_utils import run_bass_kernel_spmd
from concourse.masks import make_identity

S, B, D, H, C = 2048, 2, 1024, 16, 64
EPS = 1e-6
NCORES = 8
T = S * B                  # 4096 tokens, batch-major: t = b*S + s
LH = H // NCORES           # 2 local heads
LC = LH * C                # 128 local head columns
TCH = 512                  # phase-1 token chunk
NCHUNK = T // TCH          # 8
ICH = 1024                 # phase-2 query chunk (per batch)
TOK_OUT = T // NCORES      # 512 output tokens per core

F32 = mybir.dt.float32
F32R = mybir.dt.float32r
BF16 = mybir.dt.bfloat16
AF = mybir.ActivationFunctionType

_CACHE = {}
LAST_RESULTS = None


def r(ap):
    return ap


def _build():
    nc = bacc.Bacc("TRN2", target_bir_lowering=False, debug=False,
                   num_devices=NCORES)
    xT = nc.dram_tensor("xT", [D, T], F32, kind="ExternalInput")
    wq = nc.dram_tensor("wq", [D, LC], F32, kind="ExternalInput")
    wk = nc.dram_tensor("wk", [D, LC], F32, kind="ExternalInput")
    wv = nc.dram_tensor("wv", [D, LC], F32, kind="ExternalInput")
    wo = nc.dram_tensor("wo", [H * C, D], F32, kind="ExternalInput")
    ropeA = nc.dram_tensor("ropeA", [LC, T], F32, kind="ExternalInput")
    ropeB = nc.dram_tensor("ropeB", [LC, T], F32, kind="ExternalInput")
    qw = nc.dram_tensor("qw", [C], F32, kind="ExternalInput")
    kw = nc.dram_tensor("kw", [C], F32, kind="ExternalInput")
    out = nc.dram_tensor("out", [TOK_OUT, D], F32, kind="ExternalOutput")

    xT4 = xT.rearrange("(a p) t -> p a t", p=128)       # [128, 8, T]
    wq4 = wq.rearrange("(a p) c -> p a c", p=128)       # [128, 8, LC]
    wk4 = wk.rearrange("(a p) c -> p a c", p=128)
    wv4 = wv.rearrange("(a p) c -> p a c", p=128)
    wo4 = wo.rearrange("(a p) n -> p a n", p=128)       # [128, 8, D]

    with tile.TileContext(nc) as tc:
        with (
            tc.tile_pool(name="singles", bufs=1) as singles,
            tc.tile_pool(name="xtp", bufs=4) as xtp,
            tc.tile_pool(name="ropep", bufs=2) as ropep,
            tc.tile_pool(name="workp", bufs=3) as workp,
            tc.tile_pool(name="expp", bufs=2) as expp,
            tc.tile_pool(name="outp", bufs=2) as outp,
            tc.tile_pool(name="ps_big", bufs=2, space="PSUM") as ps_big,
            tc.tile_pool(name="ps_op", bufs=2, space="PSUM") as ps_op,
            tc.tile_pool(name="dram", bufs=1, space="DRAM") as dram,
        ):
            # ---- constants ----
            ident = singles.tile([128, 128], F32)
            make_identity(nc, ident)
            ones2f = singles.tile([128, 2], F32)
            nc.vector.memset(ones2f, 0.0)
            nc.vector.memset(ones2f[0:64, 0:1], 1.0)
            nc.vector.memset(ones2f[64:128, 1:2], 1.0)
            ones2 = singles.tile([128, 2], F32R)
            nc.vector.tensor_copy(ones2, ones2f)
            eps2 = singles.tile([2, 1], F32)
            nc.vector.memset(eps2, EPS)
            eps128 = singles.tile([128, 1], F32)
            nc.vector.memset(eps128, EPS)
            eps64 = singles.tile([128, 1], F32)
            nc.vector.memset(eps64, C * EPS)

            # ---- weights ----
            wq_sb = singles.tile([128, 8, LC], BF16)
            wk_sb = singles.tile([128, 8, LC], BF16)
            wv_sb = singles.tile([128, 8, LC], BF16)
            nc.gpsimd.dma_start(out=wq_sb, in_=wq4)
            nc.gpsimd.dma_start(out=wk_sb, in_=wk4)
            nc.gpsimd.dma_start(out=wv_sb, in_=wv4)

            # ---- persistent activations ----
            kRstd = singles.tile([128, T // 128, LH], F32)  # rstd_k/8 per token
            qw128 = singles.tile([128, 1], F32)
            nc.sync.dma_start(out=qw128[0:64, :], in_=qw[:, None])
            nc.sync.dma_start(out=qw128[64:128, :], in_=qw[:, None])
            kw128 = singles.tile([128, 1], F32)
            nc.sync.dma_start(out=kw128[0:64, :], in_=kw[:, None])
            nc.sync.dma_start(out=kw128[64:128, :], in_=kw[:, None])
            qw128 = singles.tile([128, 1], F32)
            nc.sync.dma_start(out=qw128[0:64, :], in_=qw[:, None])
            nc.sync.dma_start(out=qw128[64:128, :], in_=qw[:, None])
            qT_sb = singles.tile([128, T], BF16)          # [c_local, t]
            kTz0 = singles.tile([128, T], BF16)   # head0 rows 0:64, rest 0
            kTz1 = singles.tile([128, T], BF16)   # head1 rows 64:128, rest 0
            nc.vector.memset(kTz0[64:128, :], 0.0)
            nc.vector.memset(kTz1[0:64, :], 0.0)
            v_sb = singles.tile([128, T // 128, 2 * (C + 1)], BF16)  # [t%128, t//128, 130]
            onescol = singles.tile([128, T // 128, 1], F32)
            nc.vector.memset(onescol, 1.0)
            nc.vector.tensor_copy(v_sb[:, :, C:C + 1], onescol)
            nc.vector.tensor_copy(v_sb[:, :, 2 * C + 1:2 * C + 2], onescol)
            bounce_in = dram.tile([NCORES * 128, TOK_OUT], BF16)

            # ================= phase 1: projections + norm + rope =========
            for ch in range(NCHUNK):
                t0 = ch * TCH
                xt = xtp.tile([128, 8, TCH], BF16, tag="xt")
                if ch == 0:
                    for a8 in range(8):
                        nc.gpsimd.dma_start(out=xt[:, a8, :],
                                            in_=xT4[:, a8, t0:t0 + TCH])
                else:
                    nc.gpsimd.dma_start(out=xt, in_=xT4[:, :, t0:t0 + TCH])
                ra = ropep.tile([128, TCH], F32, tag="ra")
                rb = ropep.tile([128, TCH], F32, tag="rb")
                nc.sync.dma_start(out=ra, in_=ropeA[:, t0:t0 + TCH])
                nc.sync.dma_start(out=rb, in_=ropeB[:, t0:t0 + TCH])

                # ---- all three projections first (dense PE stream) ----
                psq = ps_big.tile([128, TCH], F32, tag="mm2")
                for a in range(8):
                    nc.tensor.matmul(psq, r(wq_sb[:, a, :]), r(xt[:, a, :]),
                                     start=(a == 0), stop=(a == 7))
                psk = ps_big.tile([128, TCH], F32, tag="mm2")
                for a in range(8):
                    nc.tensor.matmul(psk, r(wk_sb[:, a, :]), r(xt[:, a, :]),
                                     start=(a == 0), stop=(a == 7))
                psv = ps_op.tile([128, TCH], F32, tag="po")
                for a in range(8):
                    nc.tensor.matmul(psv, r(wv_sb[:, a, :]), r(xt[:, a, :]),
                                     start=(a == 0), stop=(a == 7))
                qraw = workp.tile([128, TCH], F32, tag="qraw")
                nc.scalar.copy(qraw, psq)
                kraw = workp.tile([128, TCH], F32, tag="kraw")
                nc.scalar.copy(kraw, psk)
                vt = workp.tile([128, TCH], F32, tag="vt")
                nc.scalar.copy(vt, psv)

                # ---- q: norm via sumsq matmul + DRAM-reshaped recip ----
                sq = workp.tile([128, TCH], F32R, tag="sq")
                nc.gpsimd.tensor_mul(sq, qraw, qraw)
                ms = ps_op.tile([2, TCH], F32, tag="po")
                nc.tensor.matmul(ms, r(ones2), r(sq), start=True, stop=True)
                rstd = workp.tile([2, TCH], F32, tag="rstd")
                nc.scalar.activation(rstd, ms, AF.Sqrt, bias=eps2,
                                     scale=1.0 / C)
                sq_dr = dram.tile([2, TCH], F32, tag="sq_dr", bufs=2)
                nc.sync.dma_start(out=sq_dr, in_=rstd)
                rsh = workp.tile([64, 2 * TCH // 64], F32, tag="rsh")
                nc.sync.dma_start(
                    out=rsh,
                    in_=bass.AP(tensor=sq_dr.tensor, offset=sq_dr.offset,
                                ap=[[2 * TCH // 64, 64], [1, 2 * TCH // 64]]))
                rshr = workp.tile([64, 2 * TCH // 64], F32, tag="rshr")
                nc.vector.reciprocal(rshr, rsh)
                rstd_dr = dram.tile([2, TCH], F32, tag="rstd_dr", bufs=2)
                nc.sync.dma_start(
                    out=bass.AP(tensor=rstd_dr.tensor, offset=rstd_dr.offset,
                                ap=[[2 * TCH // 64, 64], [1, 2 * TCH // 64]]),
                    in_=rshr)
                wbs = workp.tile([128, TCH], F32, tag="wbs")
                nc.sync.dma_start(
                    out=wbs,
                    in_=bass.AP(tensor=rstd_dr.tensor, offset=rstd_dr.offset,
                                ap=[[TCH, 2], [0, 64], [1, TCH]]))
                qw_s = workp.tile([128, TCH], F32, tag="qn0")
                nc.vector.tensor_scalar_mul(qw_s, qraw, qw128)
                qn = workp.tile([128, TCH], F32, tag="qn")
                nc.gpsimd.tensor_mul(qn, qw_s, wbs)
                t1 = workp.tile([128, TCH], F32, tag="t1")
                nc.vector.tensor_mul(t1, ra, qn)
                rot = workp.tile([128, TCH], F32, tag="rot")
                for g0 in (0, 64):
                    nc.sync.dma_start(out=rot[g0:g0 + 32, :],
                                      in_=qn[g0 + 32:g0 + 64, :])
                    nc.sync.dma_start(out=rot[g0 + 32:g0 + 64, :],
                                      in_=qn[g0:g0 + 32, :])
                t2 = workp.tile([128, TCH], F32, tag="t2")
                nc.gpsimd.tensor_mul(t2, rb, rot)
                qf = workp.tile([128, TCH], F32, tag="qf")
                nc.vector.tensor_add(qf, t1, t2)
                nc.scalar.copy(qT_sb[:, t0:t0 + TCH], qf)

                # ---- k: rstd_k deferred to exp scale (commutes w/ rope) ----
                sqk = workp.tile([128, TCH], F32R, tag="sqk")
                nc.gpsimd.tensor_mul(sqk, kraw, kraw)
                mst = ps_op.tile([128, 2 * (TCH // 128)], F32, tag="po")
                for s5 in range(TCH // 128):
                    nc.tensor.matmul(mst[:, s5 * 2:(s5 + 1) * 2],
                                     r(sqk[:, s5 * 128:(s5 + 1) * 128]),
                                     r(ones2), start=True, stop=True)
                srt = workp.tile([128, 2 * (TCH // 128)], F32, tag="srt")
                nc.scalar.activation(srt, mst, AF.Sqrt, bias=eps64, scale=1.0)
                nc.vector.reciprocal(
                    kRstd[:, t0 // 128:(t0 + TCH) // 128, :]
                    .rearrange("p a b -> p (a b)"), srt)
                kw_s = workp.tile([128, TCH], F32, tag="kw_s")
                nc.vector.tensor_scalar_mul(kw_s, kraw, kw128)
                t1k = workp.tile([128, TCH], F32, tag="t1")
                nc.vector.tensor_mul(t1k, ra, kw_s)
                rotk = workp.tile([128, TCH], F32, tag="rot")
                for g0 in (0, 64):
                    nc.sync.dma_start(out=rotk[g0:g0 + 32, :],
                                      in_=kw_s[g0 + 32:g0 + 64, :])
                    nc.sync.dma_start(out=rotk[g0 + 32:g0 + 64, :],
                                      in_=kw_s[g0:g0 + 32, :])
                t2k = workp.tile([128, TCH], F32, tag="t2")
                nc.gpsimd.tensor_mul(t2k, rb, rotk)
                kf = workp.tile([128, TCH], F32, tag="qf")
                nc.vector.tensor_add(kf, t1k, t2k)
                nc.scalar.copy(kTz0[0:64, t0:t0 + TCH], kf[0:64, :])
                nc.scalar.copy(kTz1[64:128, t0:t0 + TCH], kf[64:128, :])

                # ---- v: transpose to [t, c] with aug ones columns ----
                for s5 in range(TCH // 128):
                    pt = ps_op.tile([128, 128], F32, tag="po")
                    nc.tensor.transpose(pt, vt[:, s5 * 128:(s5 + 1) * 128],
                                        ident)
                    tt = (t0 + s5 * 128) // 128
                    nc.vector.tensor_copy(v_sb[:, tt, 0:C], pt[:, 0:C])
                    nc.vector.tensor_copy(v_sb[:, tt, C + 1:2 * C + 1],
                                          pt[:, C:2 * C])

            wo_a = xtp.tile([128, 8, 512], BF16, tag="xt")
            wo_b = xtp.tile([128, 8, 512], BF16, tag="xt")
            nc.gpsimd.dma_start(out=wo_a, in_=wo4[:, :, 0:512])
            nc.gpsimd.dma_start(out=wo_b, in_=wo4[:, :, 512:1024])

            # ================= phase 2: attention ========================
            for b in range(B):
                for ic in range(S // ICH):
                    q0 = b * S + ic * ICH
                    pos = [ps_op.tile([C + 1, ICH], F32, tag="po",
                                      name=f"po_{b}_{ic}_{lh}")
                           for lh in range(LH)]
                    for jt in range(S // 128):
                        j0 = b * S + jt * 128
                        pss = [ps_big.tile([128, ICH], F32, tag="mm2",
                                           name=f"pss_{b}_{ic}_{jt}_{lh}")
                               for lh in range(LH)]
                        for hf in range(2):
                            for lh in range(LH):
                                ktz = kTz0 if lh == 0 else kTz1
                                nc.tensor.matmul(
                                    pss[lh][:, hf * 512:(hf + 1) * 512],
                                    r(ktz[:, j0:j0 + 128]),
                                    r(qT_sb[:,
                                            q0 + hf * 512:q0 + (hf + 1) * 512]),
                                    start=True, stop=True)
                        exs = []
                        for lh in range(LH):
                            ex = expp.tile([128, ICH], BF16, tag="ex",
                                           name=f"ex_{b}_{ic}_{jt}_{lh}")
                            nc.scalar.activation(
                                ex, pss[lh], AF.Exp, bias=0.0,
                                scale=kRstd[:, j0 // 128, lh:lh + 1])
                            exs.append(ex)
                        for hf in range(2):
                            for lh in range(LH):
                                nc.tensor.matmul(
                                    pos[lh][:, hf * 512:(hf + 1) * 512],
                                    r(v_sb[:, j0 // 128,
                                           lh * (C + 1):(lh + 1) * (C + 1)]),
                                    r(exs[lh][:, hf * 512:(hf + 1) * 512]),
                                    start=(jt == 0), stop=(jt == S // 128 - 1))
                    for lh in range(LH):
                        hr = C * lh
                        po = pos[lh]
                        po_sb = workp.tile([C + 1, ICH], F32, tag="po_sb")
                        nc.vector.tensor_copy(po_sb, po)
                        dn_dr = dram.tile([1, ICH], F32, tag="dn_dr",
                                          bufs=2)
                        nc.sync.dma_start(out=dn_dr, in_=po_sb[C:C + 1, :])
                        dnsh = workp.tile([64, ICH // 64], F32, tag="rsh")
                        nc.sync.dma_start(
                            out=dnsh,
                            in_=bass.AP(tensor=dn_dr.tensor, offset=dn_dr.offset,
                                        ap=[[ICH // 64, 64], [1, ICH // 64]]))
                        dnshr = workp.tile([64, ICH // 64], F32, tag="rshr")
                        nc.vector.reciprocal(dnshr, dnsh)
                        den_dr = dram.tile([1, ICH], F32, tag="den_dr",
                                           bufs=2)
                        nc.sync.dma_start(
                            out=bass.AP(tensor=den_dr.tensor,
                                        offset=den_dr.offset,
                                        ap=[[ICH // 64, 64], [1, ICH // 64]]),
                            in_=dnshr)
                        nrm = workp.tile([C, ICH], F32, tag="wbs")
                        nc.sync.dma_start(
                            out=nrm,
                            in_=bass.AP(tensor=den_dr.tensor,
                                        offset=den_dr.offset,
                                        ap=[[0, C], [1, ICH]]))
                        att_tmp = workp.tile([C, ICH], BF16, tag="att_tmp")
                        nc.gpsimd.tensor_mul(att_tmp, po_sb[0:C, :], nrm)
                        for hf in range(2):
                            g2 = (q0 + hf * 512) // TOK_OUT
                            nc.sync.dma_start(
                                out=bounce_in[g2 * 128 + hr:g2 * 128 + hr + C,
                                              :],
                                in_=att_tmp[:, hf * 512:(hf + 1) * 512])

            # ================= phase 3: all-to-all + out projection =======
            bounce_out = dram.tile([NCORES * 128, TOK_OUT], BF16)
            nc.gpsimd.collective_compute(
                "AllToAll", mybir.AluOpType.bypass,
                replica_groups=[list(range(NCORES))],
                ins=[bounce_in[:, :].opt()],
                outs=[bounce_out[:, :].opt()])

            att_all = xtp.tile([128, 8, TOK_OUT], BF16, tag="xt")
            nc.sync.dma_start(
                out=att_all,
                in_=bounce_out.rearrange("(g p) t -> p g t", p=128))
            for ts in range(TOK_OUT // 128):
                out_sb = outp.tile([128, D], F32, tag="osb")
                for nh in range(2):
                    pp = ps_big.tile([128, 512], F32, tag="mm2")
                    for a in range(8):
                        nc.tensor.matmul(
                            pp, r(att_all[:, a, ts * 128:(ts + 1) * 128]),
                            r((wo_a if nh == 0 else wo_b)[:, a, :]),
                            start=(a == 0), stop=(a == 7))
                    nc.vector.tensor_copy(out_sb[:, nh * 512:(nh + 1) * 512], pp)
                nc.sync.dma_start(out=out[ts * 128:(ts + 1) * 128, :],
                                  in_=out_sb)

    nc.compile()
    return nc


def kernel(x, rope_emb, Wq, Wk, Wv, q_norm_w, k_norm_w, Wout):
    global LAST_RESULTS
    if "nc" not in _CACHE:
        _CACHE["nc"] = _build()
    nc = _CACHE["nc"]

    # batch-major tokens: t = b*S + s
    x2 = np.ascontiguousarray(
        np.transpose(np.asarray(x, np.float32), (1, 0, 2)).reshape(T, D))
    xT_np = np.ascontiguousarray(x2.T)

    re = np.asarray(rope_emb, np.float32)
    cosT = np.ascontiguousarray(re[:, :, 0, 0].T)    # [32, S]
    r01T = np.ascontiguousarray(re[:, :, 0, 1].T)
    r10T = np.ascontiguousarray(re[:, :, 1, 0].T)
    cos2 = np.concatenate([cosT, cosT], axis=1)      # [32, T] batch-major
    r01_2 = np.concatenate([r01T, r01T], axis=1)
    r10_2 = np.concatenate([r10T, r10T], axis=1)
    ropeA_np = np.ascontiguousarray(
        np.concatenate([cos2, cos2, cos2, cos2], axis=0))
    ropeB_np = np.ascontiguousarray(
        np.concatenate([r01_2, r10_2, r01_2, r10_2], axis=0))

    Wq = np.asarray(Wq, np.float32)
    Wk = np.asarray(Wk, np.float32)
    Wv = np.asarray(Wv, np.float32)
    Wout = np.ascontiguousarray(np.asarray(Wout, np.float32))
    qw_np = np.ascontiguousarray(np.asarray(q_norm_w, np.float32))
    kw_np = np.ascontiguousarray(np.asarray(k_norm_w, np.float32))

    in_maps = []
    for g in range(NCORES):
        sl = slice(g * LC, (g + 1) * LC)
        in_maps.append({
            "xT": xT_np,
            "wq": np.ascontiguousarray(Wq[:, sl]),
            "wk": np.ascontiguousarray(Wk[:, sl]),
            "wv": np.ascontiguousarray(Wv[:, sl]),
            "wo": Wout,
            "ropeA": ropeA_np,
            "ropeB": ropeB_np,
            "qw": qw_np,
            "kw": kw_np,
        })

    res = run_bass_kernel_spmd(nc, in_maps, core_ids=list(range(NCORES)))
    LAST_RESULTS = res
    out_full = np.concatenate([res.results[g]["out"] for g in range(NCORES)],
                              axis=0)                 # [T, D] batch-major
    return np.ascontiguousarray(
        out_full.reshape(B, S, D).transpose(1, 0, 2))
